# revision 14
# baseline (speedup 1.0000x reference)
"""Channel-wise FC kernel for Trainium2 (8 NeuronCores, SPMD).

Problem: out[b,c] = x[b,c] @ weights[c].T + bias[c]
  x: (8, 32, 1024, 512) f32, weights: (32, 512, 512) f32, bias: (32, 512) f32

Sharding: channel-parallel - core i owns channels [4i, 4i+4). For each channel
the device computes YT[f, bn] = sum_g WT[g,f] * XT[g, bn] (+bias); the host
does all layout transposes (free wrt HW time).

Mixed precision: the PE runs fp16 at ~216 ns per [128x128]x[128,512] matmul
and fp8e4 DoubleRow at the same wall time per matmul but 2x the MACs
(256-deep contraction). A FRACTION of the 16 per-core chunks is computed
purely in fp8: x and W quantized to e4m3 with power-of-2 scales (SX=32,
SW=128), 2 DoubleRow matmuls per output slice instead of 4 fp16 matmuls.
Per-fp8-chunk rel-err is ~3.55e-2; with n8 = 4.75 chunk-equivalents fp8 the
global rel err is sqrt(4.75/16)*3.55e-2 ~ 1.93e-2 < the 2e-2 gate.

Chunk schedule (idx = c*N_CHUNKS + nb): ALL of channel 0 (idx 0-3) is fp8 -
this skips channel 0's 2 MiB bf16 weight load entirely (only w8 needed) and
makes the head of the pipeline cheap on DMA (1 MiB x-loads) while the PE
ramps its p-state clock. idx 15 is 3/4 fp8 (ns 0-2 fp8, ns 3 fp16) so the
tail chunk drains fast. The kernel is co-bound (PE ~207us, DMA ~66 MiB at
~337 GB/s ~ 205us incl fixed ~8.7us runtime preamble).

Head: the PE warms up on an uninitialized SBUF tile (no memset dependency,
starts right at sequencer bring-up ~5.9us) while the first data DMAs land;
chunk 0's x8 arrives in 4 column-quarters so real matmuls start ~10.5us.
Loads always issue BEFORE stores within an iteration (a store's semaphore
wait blocks later FIFO entries on the sync queue); prefetch runs 3 chunks
ahead so the fp8->fp16 transition at idx 4 (w_c1 + x_4) is already in
flight before chunk 0-3's stores enter the FIFO.

Device-side DRAM layouts (host-prepped) keep every DMA reading/writing long
contiguous per-partition lines:

  xt  [C_LOC, N_CHUNKS, P, GT, NCH] f16   xt[c,nb,p,gt,n] = x^T[c, gt*128+p, nb*NCH+n]
  x8t [C_LOC, N_CHUNKS, P, GT, NCH] f8e4  = e4m3(SX * same)
  wt  [C_LOC, P, GT*F]              f16   wt[c,p,gt*F+f] = W[c, f, gt*128+p]
  w8t [C_LOC, P, GT, F]             f8e4  = e4m3(SW * same)
  bias[P, C_LOC*FT]                 f32   bias[p, c*FT+ft] = bias[c, ft*128+p]
  out [C_LOC, N_CHUNKS, P, FT*NCH]  f16   out[c,nb,p,ft*NCH+n] = Y^T[c, ft*128+p, nb*NCH+n]
"""

import os
import sys

for _p in ("/root/.axon_site/_ro/trn_rl_repo", "/opt/trn_rl_repo"):
    if os.path.isdir(_p) and _p not in sys.path:
        sys.path.append(_p)

import numpy as np

B, C, N, F, G = 8, 32, 1024, 512, 512
NCORES = 8
C_LOC = C // NCORES          # 4 channels per core
BN = B * N                   # 8192 rows per channel
P = 128
GT = G // P                  # 4 contraction tiles
FT = F // P                  # 4 output-partition tiles
NCH = 2048                   # rows per x DMA chunk
N_CHUNKS = BN // NCH         # 4
NSL = NCH // 512             # 512-row matmul slices per chunk
NIDX = C_LOC * N_CHUNKS      # 16 chunks per core

# channel 0 (idx 0-3) is entirely fp8 DoubleRow; idx 15 too (fast tail
# drain). n8 = 5 chunk-equivalents: rel err ~ sqrt(5/16)*3.55e-2 ~ 1.99e-2.
FP8_IDXS = frozenset({0, 1, 2, 3, 15})
PARTIAL_IDX = None
PARTIAL_NS = 0
WARMUP = 12                  # PE p-state ramp matmuls (512-row, ~5us; ends
                             # right as chunk 0's first columns land)
SX = 32.0                    # x fp8 scale (pow2; max|x*SX| ~ 177 < 240)
SW = 128.0                   # w fp8 scale (pow2; max|w*SW| = 128 < 240)
SINV = 1.0 / (SX * SW)

_F16 = np.float16
try:
    import ml_dtypes
    _F8 = ml_dtypes.float8_e4m3  # neuron float8e4 (max normal 240)
except ImportError:  # pragma: no cover
    _F8 = None

_compiled = None


def _build():
    import concourse.bacc as bacc
    import concourse.mybir as mybir
    import concourse.tile as tile

    F16 = mybir.dt.float16
    F8 = mybir.dt.float8e4
    F32 = mybir.dt.float32
    DR = mybir.MatmulPerfMode.DoubleRow

    nc = bacc.Bacc("TRN2", target_bir_lowering=False, debug=False)
    xt = nc.dram_tensor("xt", [C_LOC, N_CHUNKS, P, GT, NCH], F16,
                        kind="ExternalInput")
    x8t = nc.dram_tensor("x8t", [C_LOC, N_CHUNKS, P, GT, NCH], F8,
                         kind="ExternalInput")
    wt = nc.dram_tensor("wt", [C_LOC, P, GT * F], F16, kind="ExternalInput")
    w8t = nc.dram_tensor("w8t", [C_LOC, P, GT, F], F8, kind="ExternalInput")
    bias = nc.dram_tensor("bias", [P, C_LOC * FT], F32, kind="ExternalInput")
    out = nc.dram_tensor("out", [C_LOC, N_CHUNKS, P, FT * NCH], F16,
                         kind="ExternalOutput")

    xt_ap = xt.ap()
    x8t_ap = x8t.ap()
    wt_ap = wt.ap()
    w8t_ap = w8t.ap()
    out_ap = out.ap()

    # which chunks are pure fp8 / which channels never need fp16 weights
    def is_f8(idx):
        return idx in FP8_IDXS

    f16_chunk_idxs = [i for i in range(NIDX)
                      if not is_f8(i)]
    # channels needing fp16 weights (any non-pure-fp8 chunk)
    w_channels = sorted({i // N_CHUNKS for i in f16_chunk_idxs})
    assert 0 not in w_channels, "channel 0 must be all-fp8 (skips its w load)"

    with tile.TileContext(nc) as tc:
        with (
            tc.tile_pool(name="wpool", bufs=2) as wpool,
            tc.tile_pool(name="w8pool", bufs=2) as w8pool,
            tc.tile_pool(name="xpool", bufs=4) as xpool,
            tc.tile_pool(name="x8pool", bufs=3) as x8pool,
            tc.tile_pool(name="opool", bufs=4) as opool,
            tc.tile_pool(name="bpool", bufs=1) as bpool,
            tc.tile_pool(name="warmpool", bufs=1) as warmpool,
            tc.tile_pool(name="psum", bufs=8, space="PSUM") as pspool,
        ):
            # --- PE warmup burst, initialized by the otherwise-idle GpSimd
            # engine (alive ~5.8us, well before Vector): short 128-row
            # matmuls ramp the p-state clock (~3us of PE busy) while the
            # first data DMAs are still in flight, so the first real matmul
            # fires the moment its data lands.
            warm_sb = warmpool.tile([P, 512], F16)
            nc.gpsimd.memset(warm_sb[:], 0.0)
            warm_ps = pspool.tile([P, 512], F32, tag="ps")
            for _ in range(WARMUP):
                nc.tensor.matmul(warm_ps[:], warm_sb[:, :P], warm_sb[:],
                                 start=True, stop=True)

            def evict(c, ft, src, dst, f8, act):
                bcol = b_sb[:, c * FT + ft:c * FT + ft + 1]
                if act:
                    nc.scalar.activation(
                        dst, src, mybir.ActivationFunctionType.Identity,
                        bias=bcol, scale=(SINV if f8 else 1.0),
                    )
                elif f8:
                    nc.vector.tensor_scalar(
                        out=dst, in0=src, scalar1=SINV, scalar2=bcol,
                        op0=mybir.AluOpType.mult, op1=mybir.AluOpType.add,
                    )
                else:
                    nc.vector.tensor_scalar_add(dst, src, bcol)

            # xcol: column base of the 512-row slice within the x tile;
            # ns: destination slice index within the chunk
            def mm_group(c, nb, ns, ft, w_sb, x_sb, xcol, o_sb, act):
                ps = pspool.tile([P, 512], F32, tag="ps",
                                 name=f"ps_{c}_{nb}_{ns}_{ft}")
                for gt in range(GT):
                    nc.tensor.matmul(
                        ps[:],
                        w_sb[:, gt * F + ft * P:gt * F + (ft + 1) * P],
                        x_sb[:, gt, xcol:xcol + 512],
                        start=(gt == 0),
                        stop=(gt == GT - 1),
                    )
                evict(c, ft, ps[:],
                      o_sb[:, ft * NCH + ns * 512:ft * NCH + (ns + 1) * 512],
                      False, act)

            def mm_group_f8(c, nb, ns, ft, w8_sb, x8_sb, xcol, o_sb, act):
                ps = pspool.tile([P, 512], F32, tag="ps",
                                 name=f"ps8_{c}_{nb}_{ns}_{ft}")
                for h in range(2):
                    nc.tensor.matmul(
                        ps[:],
                        w8_sb[:, 2 * h:2 * h + 2, ft * P:(ft + 1) * P],
                        x8_sb[:, 2 * h:2 * h + 2, xcol:xcol + 512],
                        start=(h == 0),
                        stop=(h == 1),
                        perf_mode=DR,
                    )
                evict(c, ft, ps[:],
                      o_sb[:, ft * NCH + ns * 512:ft * NCH + (ns + 1) * 512],
                      True, act)

            b_sb = bpool.tile([P, C_LOC * FT], F32)
            w_sbs, w8_sbs, x_sbs = {}, {}, {}
            xp_sbs = {}              # PARTIAL_IDX: (x8 part, x16 part)

            # everything stays on the SP (sync) HWDGE queue: the Activation
            # HWDGE queue's transfers proved ~5x slower on HW
            def load_w(c):
                w_sbs[c] = wpool.tile([P, GT * F], F16, tag="w", name=f"w_{c}")
                nc.sync.dma_start(w_sbs[c][:], wt_ap[c])

            def load_w8(c):
                w8_sbs[c] = w8pool.tile([P, GT, F], F8, tag="w8",
                                        name=f"w8_{c}")
                nc.sync.dma_start(w8_sbs[c][:], w8t_ap[c])

            def load_x(idx, split=1):
                c, nb = divmod(idx, N_CHUNKS)
                if idx == PARTIAL_IDX:
                    if c not in w8_sbs:
                        load_w8(c)
                    pcols = PARTIAL_NS * 512
                    x8p = x8pool.tile([P, GT, pcols], F8, tag="x8p",
                                      name=f"x8p_{c}_{nb}", bufs=1)
                    x16p = xpool.tile([P, GT, NCH - pcols], F16, tag="xp",
                                      name=f"xp_{c}_{nb}", bufs=1)
                    nc.sync.dma_start(x8p[:], x8t_ap[c, nb][:, :, :pcols])
                    nc.sync.dma_start(x16p[:], xt_ap[c, nb][:, :, pcols:])
                    xp_sbs[idx] = (x8p, x16p)
                elif is_f8(idx):
                    if c not in w8_sbs:
                        load_w8(c)
                    x_sbs[idx] = x8pool.tile([P, GT, NCH], F8, tag="x8",
                                             name=f"x8_{c}_{nb}")
                    nc.sync.dma_start(x_sbs[idx][:], x8t_ap[c, nb])
                else:
                    x_sbs[idx] = xpool.tile([P, GT, NCH], F16, tag="x",
                                            name=f"x_{c}_{nb}")
                    if split == 1:
                        nc.sync.dma_start(x_sbs[idx][:], xt_ap[c, nb])
                    else:
                        step = NCH // split
                        for s in range(split):
                            lo = s * step
                            nc.sync.dma_start(
                                x_sbs[idx][:, :, lo:lo + step],
                                xt_ap[c, nb][:, :, lo:lo + step],
                            )

            # --- head: chunk 0's w8 + x8 interleaved in DoubleRow
            # consumption order, leading with ns-slice-0's columns so the
            # first real matmul group can fire after ~1 MiB has landed.
            # bias rides AFTER them (it is only needed by the first
            # eviction, and its trigger would cost 600ns at the very head).
            w8_sbs[0] = w8pool.tile([P, GT, F], F8, tag="w8", name="w8_0")
            x_sbs[0] = x8pool.tile([P, GT, NCH], F8, tag="x8", name="x8_0_0")
            # (each dma_start costs ~600ns of descriptor-generation on the
            # sync sequencer, so the head stays at 6 pieces -- finer bites
            # delay chunk 1's trigger more than they advance chunk 0)
            nc.sync.dma_start(w8_sbs[0][:, 0:2, :], w8t_ap[0][:, 0:2, :])
            nc.sync.dma_start(x_sbs[0][:, 0:2, :1024],
                              x8t_ap[0, 0][:, 0:2, :1024])
            nc.sync.dma_start(w8_sbs[0][:, 2:4, :], w8t_ap[0][:, 2:4, :])
            nc.sync.dma_start(x_sbs[0][:, 2:4, :1024],
                              x8t_ap[0, 0][:, 2:4, :1024])
            nc.sync.dma_start(x_sbs[0][:, 0:2, 1024:],
                              x8t_ap[0, 0][:, 0:2, 1024:])
            nc.sync.dma_start(x_sbs[0][:, 2:4, 1024:],
                              x8t_ap[0, 0][:, 2:4, 1024:])
            nc.sync.dma_start(b_sb[:], bias.ap())
            load_x(1)

            # loads issued at the TOP of iteration idx (before any stores of
            # this iteration hit the FIFO). (channel, split) derived inline.
            LOAD_SCHED = {
                0: [2],
                1: [3, 4],           # w_c1 + x_4 queued before o_0's store
                2: [5],
                3: [6],
                4: [7],
                5: [8],              # w_c2 before x_8
                6: [9],
                7: [10],
                8: [11],
                9: [12],             # w_c3 before x_12
                10: [13],
                11: [14],
                12: [15],            # w8_c3 + partial x8/x16
            }
            # split the first two fp16 x loads after the fp8 block so the
            # PE can start on partial chunks (finer DMA semaphores)
            SPLITS = {4: 4, 5: 2}

            def emit_group(idx, c, nb, ns, ft, o_sb, act):
                if idx == PARTIAL_IDX:
                    x8p, x16p = xp_sbs[idx]
                    if ns < PARTIAL_NS:
                        mm_group_f8(c, nb, ns, ft, w8_sbs[c], x8p,
                                    ns * 512, o_sb, act)
                    else:
                        mm_group(c, nb, ns, ft, w_sbs[c], x16p,
                                 (ns - PARTIAL_NS) * 512, o_sb, act)
                elif is_f8(idx):
                    mm_group_f8(c, nb, ns, ft, w8_sbs[c], x_sbs[idx],
                                ns * 512, o_sb, act)
                else:
                    mm_group(c, nb, ns, ft, w_sbs[c], x_sbs[idx],
                             ns * 512, o_sb, act)

            # Chunks 0..NIDX-3 run ns-outer with ONE full-chunk store at the
            # end (16KB/partition descriptors -- best DMA efficiency; earlier
            # per-ft stores would steal bandwidth from the x prefetches).
            # The LAST TWO chunks run ft-outer with per-ft stores so the
            # output flushes during compute and the post-matmul drain is
            # small -- by then there are no more loads to starve.
            for idx in range(NIDX):
                c, nb = divmod(idx, N_CHUNKS)
                for nxt in LOAD_SCHED.get(idx, ()):
                    nxt_c = nxt // N_CHUNKS
                    if (nxt not in FP8_IDXS and nxt != PARTIAL_IDX
                            and nxt_c not in w_sbs):
                        load_w(nxt_c)
                    load_x(nxt, split=SPLITS.get(nxt, 1))
                o_sb = opool.tile([P, FT * NCH], F16, tag="o",
                                  name=f"o_{c}_{nb}")
                if idx < NIDX - 2:
                    for ns in range(NSL):
                        for ft in range(FT):
                            emit_group(idx, c, nb, ns, ft, o_sb, ft % 2 == 0)
                    nc.sync.dma_start(out_ap[c, nb], o_sb[:])
                else:
                    last = idx == NIDX - 1
                    for ft in range(FT):
                        fin = last and ft == FT - 1
                        for ns in range(NSL):
                            # the very last eviction goes to ACT (687ns vs
                            # DVE's 751ns), hence the flipped parity on the
                            # final ft
                            emit_group(idx, c, nb, ns, ft, o_sb,
                                       ns % 2 == (1 if fin else 0))
                            if fin and ns >= 1:
                                # final ft: flush in 3 pieces (1024/512/512
                                # cols) so only one small store trails the
                                # last matmul and few sync-queue triggers
                                # serialize at the drain
                                lo = ft * NCH + (0 if ns == 1 else ns * 512)
                                hi = ft * NCH + (ns + 1) * 512
                                nc.sync.dma_start(
                                    out_ap[c, nb][:, lo:hi],
                                    o_sb[:, lo:hi],
                                )
                        if not fin:
                            lo = ft * NCH
                            nc.sync.dma_start(
                                out_ap[c, nb][:, lo:lo + NCH],
                                o_sb[:, lo:lo + NCH],
                            )
    nc.compile()
    return nc


def _get_compiled():
    global _compiled
    if _compiled is None:
        _compiled = _build()
    return _compiled


def _shard_inputs(x, weights, bias):
    """Host-side: slice channels per core, cast (fp16 + scaled-fp8), and
    pre-transpose into the device DRAM layouts documented at the top."""
    x = np.asarray(x, dtype=np.float32)
    weights = np.asarray(weights, dtype=np.float32)
    bias = np.asarray(bias, dtype=np.float32)

    # (B, C, N, G) -> (C, G, B*N) -> (C, GT, P, N_CHUNKS, NCH) -> (C, nb, p, gt, n)
    xt_f32 = (
        x.transpose(1, 3, 0, 2)
        .reshape(C, GT, P, N_CHUNKS, NCH)
        .transpose(0, 3, 2, 1, 4)
    )
    xt_all = xt_f32.astype(_F16)                  # (C, nb, P, gt, n)
    x8_all = (xt_f32 * SX).astype(_F8)            # (C, nb, P, gt, n)
    # (C, F, G) -> W^T (C, G, F) -> (C, GT, P, F) -> (C, p, gt, F)
    wt_f32 = (
        weights.transpose(0, 2, 1)
        .reshape(C, GT, P, F)
        .transpose(0, 2, 1, 3)
    )
    wt_all = wt_f32.reshape(C, P, GT * F).astype(_F16)
    w8_all = (wt_f32 * SW).astype(_F8)            # (C, p, gt, F)
    # (C, F) -> (C, FT, P) -> (P, C, FT)
    bias_all = (
        bias.reshape(C, FT, P).transpose(2, 0, 1).reshape(P, C * FT)
        .astype(np.float32)
    )

    in_maps = []
    for i in range(NCORES):
        sl = slice(i * C_LOC, (i + 1) * C_LOC)
        in_maps.append({
            "xt": np.ascontiguousarray(xt_all[sl]),
            "x8t": np.ascontiguousarray(x8_all[sl]),
            "wt": np.ascontiguousarray(wt_all[sl]),
            "w8t": np.ascontiguousarray(w8_all[sl]),
            "bias": np.ascontiguousarray(
                bias_all[:, i * C_LOC * FT:(i + 1) * C_LOC * FT]
            ),
        })
    return in_maps


def _unshard_output(results):
    # per-core out: (C_LOC, N_CHUNKS, P, FT*NCH) f16
    yt = np.stack([np.asarray(r["out"]) for r in results])
    # (NCORES, C_LOC, nb, p, ft, n) -> (C, ft, p, nb, n) == (C, F, BN)
    yt = (
        yt.reshape(C, N_CHUNKS, P, FT, NCH)
        .transpose(0, 3, 2, 1, 4)
        .reshape(C, F, B, N)
    )
    y = yt.transpose(2, 0, 3, 1).astype(np.float32)  # (B, C, N, F)
    return np.ascontiguousarray(y)


def _ensure_axon_hooks():
    """bass_utils hard-imports antenv.axon_hooks when tracing is requested;
    some images lack that module. Shim it (with the ctypes NTFF hook when
    available) only if the real module is absent."""
    try:
        import antenv.axon_hooks  # noqa: F401
        return
    except ImportError:
        pass
    import types

    import antenv

    mod = types.ModuleType("antenv.axon_hooks")
    _hook = [None]
    mod.set_axon_ntff_profile_hook = lambda h: _hook.__setitem__(0, h)
    mod.get_axon_ntff_profile_hook = lambda: _hook[0]
    sys.modules["antenv.axon_hooks"] = mod
    antenv.axon_hooks = mod
    try:
        from trn_agent_boot.trn_boot import _ntff_profile_via_ctypes

        mod.set_axon_ntff_profile_hook(
            _ntff_profile_via_ctypes("/opt/axon/libaxon_pjrt.so")
        )
    except Exception:
        pass


def run_on_device(in_maps, **kwargs):
    _ensure_axon_hooks()
    from concourse.bass_utils import run_bass_kernel_spmd

    nc = _get_compiled()
    return run_bass_kernel_spmd(nc, in_maps, core_ids=list(range(NCORES)), **kwargs)


def kernel(x, weights, bias):
    in_maps = _shard_inputs(x, weights, bias)
    res = run_on_device(in_maps)
    return _unshard_output(res.results)
